# revision 5
# baseline (speedup 1.0000x reference)
"""GNN message-passing net on 8 Trainium2 cores.

Reference: x:[256,784,1] -> h1 = elu(spmm(x)@W1+b1) -> h2 = elu(spmm(h1)@W2+b2)
-> flat[B, N*C] -> relu(flat@Wf1+bf1) -> softmax(z@Wf2+bf2).

Strategy:
  * Densify the sparse filter A (784x784, ~1% nz) on the host; spmm becomes
    dense fp32r matmuls (fp32 storage, 1 cycle/row at free>=256).
  * F=1 makes conv1 an outer product: out1 = A @ X^T [784,256] shared by all
    channels; h1_c = elu(W1[c]*out1+b1[c]) via ACT Exp/Relu with fused
    per-partition scale/bias + 2 DVE ops (elu(t)=min(exp(t),1)+relu(t)-1).
  * Channel shard conv2: core k computes out2_c = A @ h1_c for channels
    4k..4k+3, full batch (free=256).
  * AllToAll (3.2MB/rank) reshards channel->batch: each core gets all 32
    pre-mix channels for its 32-batch block, packed [(ng,c),nsub,b] with the
    node dim split 4x196 across partitions.
  * W2 channel mix as one 128x128 stationary kron(I4,W2) matmul; +b2, elu.
  * FC1 batch-sharded in bf16 with Wf1 streamed from HBM (12.8MB, overlapped
    with the conv phases); +bf1, relu, FC2 (+bf2 via ones-row matmul),
    softmax.  Core k returns output rows [32k:32k+32).
"""
import json

import numpy as np

import concourse.bass as bass
import concourse.mybir as mybir
import concourse.tile as tile
from concourse.bass_utils import run_bass_kernel_spmd

B, N, F, E = 256, 784, 1, 6272
C, H, N_OUT = 32, 512, 10
NCORE = 8
CPC = C // NCORE      # 4 channels per core in conv2
P = 112               # 784 = 7 * 112
KN = N // P           # 7 node chunks
NG = 4                # node groups packed into partitions for the mix
NSUB = N // NG        # 196 nodes per group
BPC = B // NCORE      # 32 batch rows per core
HJ = H // 128         # 4 h chunks

f32 = mybir.dt.float32
f32r = mybir.dt.float32r
bf16 = mybir.dt.bfloat16
AF = mybir.ActivationFunctionType
ALU = mybir.AluOpType
AX = mybir.AxisListType


# ---------------------------------------------------------------------------
# BIR post-pass: this walrus build rejects instructions with >1 sync-wait;
# split extras onto standalone EventSemaphore instructions (same engine,
# inserted just before, so the engine stream stalls identically).
def _split_waits(bir: dict, max_waits: int = 1) -> dict:
    n = [0]
    for fn in bir.get("functions", []):
        for blk in fn.get("blocks", []):
            out = []
            for ins in blk.get("instructions", []):
                si = ins.get("sync_info") or {}
                waits = si.get("on_wait") or []
                if len(waits) > max_waits:
                    for w in waits[max_waits:]:
                        n[0] += 1
                        out.append({
                            "name": f"I-waitsplit-{n[0]}",
                            "opcode": "EventSemaphore",
                            "engine": ins["engine"],
                            "ins": [], "outs": [],
                            **({"debug": ins["debug"]} if "debug" in ins else {}),
                            "sync_info": {"on_update": [], "on_wait": [w]},
                        })
                    si = dict(si)
                    si["on_wait"] = waits[:max_waits]
                    ins = dict(ins)
                    ins["sync_info"] = si
                out.append(ins)
            blk["instructions"] = out
    return bir


def _install_wait_splitter(nc):
    orig = nc.to_json_bytes
    nc.to_json_bytes = lambda: json.dumps(_split_waits(json.loads(orig()))).encode()


# ---------------------------------------------------------------------------
def _build_program():
    nc = bass.Bass(num_devices=NCORE)

    at_d = nc.dram_tensor("at", [P, KN * N], f32r, kind="ExternalInput")
    xt_d = nc.dram_tensor("xt", [P, KN * B], f32r, kind="ExternalInput")
    wf1_d = nc.dram_tensor("wf1", [NSUB * 128, H], bf16, kind="ExternalInput")
    wb_d = nc.dram_tensor("wb", [1, 2 * CPC], f32, kind="ExternalInput")
    w2k_d = nc.dram_tensor("w2k", [128, 128], f32r, kind="ExternalInput")
    b2k_d = nc.dram_tensor("b2k", [128, 1], f32, kind="ExternalInput")
    bf1_d = nc.dram_tensor("bf1", [128, HJ], f32, kind="ExternalInput")
    wf2_d = nc.dram_tensor("wf2", [128, HJ * N_OUT], f32, kind="ExternalInput")
    bf2_d = nc.dram_tensor("bf2", [1, N_OUT], f32, kind="ExternalInput")
    out_d = nc.dram_tensor("out", [BPC, N_OUT], f32, kind="ExternalOutput")

    with tile.TileContext(nc) as tc:
        with (
            tc.tile_pool(name="big", bufs=1) as big,
            tc.tile_pool(name="small", bufs=1) as small,
            tc.tile_pool(name="ework", bufs=4) as ework,
            tc.tile_pool(name="rwork", bufs=4) as rwork,
            tc.tile_pool(name="wstream", bufs=16) as wstream,
            tc.tile_pool(name="ps_a", bufs=2, space="PSUM") as ps_a,
            tc.tile_pool(name="ps_b", bufs=2, space="PSUM") as ps_b,
            tc.tile_pool(name="ps_fc1", bufs=4, space="PSUM") as ps_fc1,
            tc.tile_pool(name="dram", bufs=1, space="DRAM") as dram,
        ):
            # ---- resident inputs ------------------------------------------
            at_sb = big.tile([P, KN, N], f32r)
            xt_sb = big.tile([P, KN, B], f32r)
            wb_sb = small.tile([1, 2 * CPC], f32)
            w2k_sb = small.tile([128, 128], f32r)
            b2k_sb = small.tile([128, 1], f32)
            bf1_sb = small.tile([128, HJ], f32)
            wf2_sb = small.tile([128, HJ, N_OUT], f32)
            bf2_sb = small.tile([1, N_OUT], f32)
            ones = small.tile([1, 128], f32)

            nc.sync.dma_start(xt_sb[:], xt_d[:].rearrange("p (k b) -> p k b", k=KN))
            nc.sync.dma_start(at_sb[:], at_d[:].rearrange("p (k n) -> p k n", k=KN))
            nc.sync.dma_start(wb_sb[:], wb_d[:])
            nc.sync.dma_start(w2k_sb[:], w2k_d[:])
            nc.sync.dma_start(b2k_sb[:], b2k_d[:])
            nc.sync.dma_start(bf1_sb[:], bf1_d[:])
            nc.sync.dma_start(wf2_sb[:], wf2_d[:].rearrange("p (j o) -> p j o", j=HJ))
            nc.sync.dma_start(bf2_sb[:], bf2_d[:])
            nc.vector.memset(ones[:], 1.0)

            # ---- broadcast W1/b1 channel scalars across partitions --------
            ps_bc = ps_a.tile([128, 512], f32, tag="psa")
            nc.tensor.matmul(ps_bc[:, 0:2 * CPC], ones[0:1, 0:128], wb_sb[:])
            wband = small.tile([128, 2 * CPC], f32)
            nc.vector.tensor_copy(wband[:], ps_bc[:, 0:2 * CPC])

            # ---- conv1 + elu ----------------------------------------------
            h1_sb = big.tile([P, CPC, KN, B], f32r)
            for mc in range(KN):
                o1 = ps_a.tile([128, 512], f32, tag="psa")
                for kc in range(KN):
                    nc.tensor.matmul(
                        o1[0:P, 0:B],
                        at_sb[:, kc, mc * P:(mc + 1) * P],
                        xt_sb[:, kc, :],
                        start=(kc == 0), stop=(kc == KN - 1),
                    )
                for c in range(CPC):
                    sc = wband[0:P, c:c + 1]
                    bi = wband[0:P, CPC + c:CPC + c + 1]
                    e = ework.tile([128, 512], f32)
                    nc.scalar.activation(e[0:P, 0:B], o1[0:P, 0:B], AF.Exp,
                                         bias=bi, scale=sc)
                    r = rwork.tile([128, 512], f32)
                    nc.scalar.activation(r[0:P, 0:B], o1[0:P, 0:B], AF.Relu,
                                         bias=bi, scale=sc)
                    nc.vector.tensor_scalar(e[0:P, 0:B], e[0:P, 0:B],
                                            1.0, -1.0, ALU.min, ALU.add)
                    nc.vector.tensor_tensor(h1_sb[:, c, mc, :], e[0:P, 0:B],
                                            r[0:P, 0:B], ALU.add)

            # ---- conv2 spmm (channel shard), straight to the A2A buffer ---
            a2a_in = dram.tile([NCORE, CPC * N, BPC], f32)
            a2a_out = dram.tile([NCORE, CPC * N, BPC], f32)
            for c in range(CPC):
                for mo in range(KN):
                    o2 = ps_b.tile([P, B], f32, tag="psb")
                    for kc in range(KN):
                        nc.tensor.matmul(
                            o2[:],
                            at_sb[:, kc, mo * P:(mo + 1) * P],
                            h1_sb[:, c, kc, :],
                            start=(kc == 0), stop=(kc == KN - 1),
                        )
                    ev = ework.tile([128, 512], f32, tag="evict")
                    if (c * KN + mo) % 2 == 0:
                        nc.vector.tensor_copy(ev[0:P, 0:B], o2[:])
                    else:
                        nc.scalar.copy(ev[0:P, 0:B], o2[:])
                    r0 = c * N + mo * P
                    for j in range(NCORE):
                        nc.sync.dma_start(
                            a2a_in[j, r0:r0 + P, :],
                            ev[0:P, j * BPC:(j + 1) * BPC],
                        )

            nc.gpsimd.collective_compute(
                "AllToAll", ALU.bypass,
                replica_groups=[list(range(NCORE))],
                ins=[a2a_in.opt()], outs=[a2a_out.opt()],
            )

            # ---- repack: rows (c, n=ng*196+s) -> [(ng,c) part, s, b] ------
            r_sb = big.tile([128, NSUB, BPC], f32r)
            # received row index (c*784 + ng*196 + s) -> partition ng*32+c
            ap = a2a_out[:].rearrange("k (cl g s) b -> (k cl) g s b",
                                      g=NG, cl=CPC)
            for ng in range(NG):
                nc.sync.dma_start(
                    r_sb[ng * C:(ng + 1) * C, :, :],
                    ap[:, ng, :, :].bitcast(f32r),
                )

            # ---- W2 mix + b2 + elu -> h2 (bf16) ---------------------------
            h2_sb = big.tile([128, NSUB, BPC], bf16)
            nslc = [(s, min(16, NSUB - s)) for s in range(0, NSUB, 16)]
            for s0, sw in nslc:
                fw = sw * BPC
                pm = ps_a.tile([128, 512], f32, tag="psa")
                nc.tensor.matmul(pm[:, 0:fw], w2k_sb[:],
                                 r_sb[:, s0:s0 + sw, :])
                e = ework.tile([128, 512], f32)
                nc.scalar.activation(e[:, 0:fw], pm[:, 0:fw], AF.Exp,
                                     bias=b2k_sb[:, 0:1])
                r = rwork.tile([128, 512], f32)
                nc.scalar.activation(r[:, 0:fw], pm[:, 0:fw], AF.Relu,
                                     bias=b2k_sb[:, 0:1])
                nc.vector.tensor_scalar(e[:, 0:fw], e[:, 0:fw],
                                        1.0, -1.0, ALU.min, ALU.add)
                nc.vector.tensor_tensor(h2_sb[:, s0:s0 + sw, :], e[:, 0:fw],
                                        r[:, 0:fw], ALU.add)

            # ---- FC1 (batch shard, bf16, Wf1 streamed) --------------------
            zps = [ps_fc1.tile([128, BPC], f32, tag="psfc1", name=f"zp{h}")
                   for h in range(HJ)]
            for s in range(NSUB):
                wt = wstream.tile([128, H], bf16)
                nc.sync.dma_start(wt[:], wf1_d[s * 128:(s + 1) * 128, :])
                for hj in range(HJ):
                    nc.tensor.matmul(
                        zps[hj][:],
                        wt[:, hj * 128:(hj + 1) * 128],
                        h2_sb[:, s, :],
                        start=(s == 0), stop=(s == NSUB - 1),
                    )

            # ---- +bf1, relu, FC2, +bf2, softmax ---------------------------
            zr_sb = small.tile([128, HJ, BPC], f32)
            for hj in range(HJ):
                nc.scalar.activation(zr_sb[:, hj, :], zps[hj][:], AF.Relu,
                                     bias=bf1_sb[:, hj:hj + 1])

            ps_o = ps_b.tile([BPC, N_OUT], f32, tag="psb")
            for hj in range(HJ):
                nc.tensor.matmul(ps_o[:], zr_sb[:, hj, :], wf2_sb[:, hj, :],
                                 start=(hj == 0), stop=False)
            nc.tensor.matmul(ps_o[:], ones[0:1, 0:BPC], bf2_sb[:],
                             start=False, stop=True)

            mx = small.tile([BPC, 1], f32)
            nc.vector.tensor_reduce(mx[:], ps_o[:], axis=AX.X, op=ALU.max,
                                    negate=True)
            t = small.tile([BPC, N_OUT], f32)
            nc.vector.tensor_scalar(t[:], ps_o[:], mx[0:BPC, 0:1], None, ALU.add)
            ex = small.tile([BPC, N_OUT], f32)
            nc.scalar.activation(ex[:], t[:], AF.Exp)
            sm = small.tile([BPC, 1], f32)
            nc.vector.tensor_reduce(sm[:], ex[:], axis=AX.X, op=ALU.add)
            rc = small.tile([BPC, 1], f32)
            nc.vector.reciprocal(rc[:], sm[:])
            ob = small.tile([BPC, N_OUT], f32)
            nc.vector.tensor_scalar(ob[:], ex[:], rc[0:BPC, 0:1], None, ALU.mult)
            nc.sync.dma_start(out_d[:], ob[:])

    _install_wait_splitter(nc)
    return nc


_NC_CACHE = None


def _get_program():
    global _NC_CACHE
    if _NC_CACHE is None:
        _NC_CACHE = _build_program()
    return _NC_CACHE


# ---------------------------------------------------------------------------
def _prep_inputs(x, edge_row, edge_col, edge_val, W1, b1, W2, b2,
                 Wf1, bf1, Wf2, bf2):
    import ml_dtypes
    f = np.float32
    A = np.zeros((N, N), f)
    np.add.at(A, (np.asarray(edge_row), np.asarray(edge_col)),
              np.asarray(edge_val, f))
    AT = np.ascontiguousarray(A.T)                                  # [m, n]
    at = np.ascontiguousarray(
        AT.reshape(KN, P, N).transpose(1, 0, 2).reshape(P, KN * N))

    XT = np.ascontiguousarray(np.asarray(x, f)[:, :, 0].T)          # [N, B]
    xt = np.ascontiguousarray(
        XT.reshape(KN, P, B).transpose(1, 0, 2).reshape(P, KN * B))

    W1 = np.asarray(W1, f); b1 = np.asarray(b1, f)
    W2 = np.asarray(W2, f); b2 = np.asarray(b2, f)
    Wf1 = np.asarray(Wf1, f); bf1 = np.asarray(bf1, f)
    Wf2 = np.asarray(Wf2, f); bf2 = np.asarray(bf2, f)

    # mix weight: lhsT[(ng,c),(ng',c')] = delta(ng,ng') * W2[c,c']
    w2k = np.kron(np.eye(NG, dtype=f), W2).astype(f)                # [128,128]
    b2k = np.tile(b2, NG).reshape(128, 1).astype(f)

    # FC1 weights, K-chunk s holds flat rows (n=ng*196+s)*C + c' in
    # partition order p = ng*C + c'.
    wf1_l = Wf1.reshape(NG, NSUB, C, H).transpose(1, 0, 2, 3).reshape(
        NSUB * 128, H).astype(ml_dtypes.bfloat16)

    bf1_l = np.ascontiguousarray(bf1.reshape(HJ, 128).T)            # [128, HJ]
    wf2_l = np.ascontiguousarray(
        Wf2.reshape(HJ, 128, N_OUT).transpose(1, 0, 2).reshape(128, HJ * N_OUT))
    bf2_l = bf2.reshape(1, N_OUT).copy()

    in_maps = []
    for k in range(NCORE):
        wb = np.concatenate([W1[0, k * CPC:(k + 1) * CPC],
                             b1[k * CPC:(k + 1) * CPC]]).reshape(1, 2 * CPC)
        in_maps.append({
            "at": at, "xt": xt, "wf1": wf1_l,
            "wb": np.ascontiguousarray(wb.astype(f)),
            "w2k": w2k, "b2k": b2k,
            "bf1": bf1_l, "wf2": wf2_l, "bf2": bf2_l,
        })
    return in_maps


def kernel(x, edge_row, edge_col, edge_val, W1, b1, W2, b2,
           Wf1, bf1, Wf2, bf2, **kw):
    nc = _get_program()
    in_maps = _prep_inputs(x, edge_row, edge_col, edge_val, W1, b1, W2, b2,
                           Wf1, bf1, Wf2, bf2)
    res = run_bass_kernel_spmd(nc, in_maps, list(range(NCORE)),
                               **kw)
    out = np.concatenate([res.results[k]["out"] for k in range(NCORE)], axis=0)
    if kw.get("trace"):
        kernel.last_exec_time_ns = res.exec_time_ns
    return out.astype(np.float32)


# revision 8
# speedup vs baseline: 2.2159x; 2.2159x over previous
"""GNN message-passing net on 8 Trainium2 cores.

Reference: x:[256,784,1] -> h1 = elu(spmm(x)@W1+b1) -> h2 = elu(spmm(h1)@W2+b2)
-> flat[B, N*C] -> relu(flat@Wf1+bf1) -> softmax(z@Wf2+bf2).

Strategy (all matmul operands bf16, fp32 PSUM accumulation):
  * Densify the sparse filter A (784x784, ~1% nz) on the host; spmm becomes
    dense matmuls on the PE array.
  * F=1 makes conv1 an outer product: out1 = A @ X^T [784,256] shared by all
    channels; h1_c = elu(W1[c]*out1+b1[c]) via ACT Exp/Relu with fused
    per-partition scale/bias + 2 DVE ops (elu(t)=min(exp(t),1)+relu(t)-1).
  * Conv2 spmm channel-sharded: core k computes out2_c = A @ h1_c for
    channels 4k..4k+3, full batch (free=256).
  * AllToAll reshards channel->node: core j receives all 32 pre-mix channels
    for nodes [112j, 112j+112) (core 7 gets zero-padded nodes 784..895),
    packed [(ng,c) partitions, s, b] with its nodes split 4x28.
  * W2 channel mix as a 128x128 stationary kron(I4,W2) matmul; +b2, elu.
  * FC1 stays K-sharded: core k holds Wf1 rows for its nodes (zero rows for
    pad nodes), 28 K-chunks x 4 h-chunks, free=256.  z^T partials [512,256]
    are ReduceScattered; each core then does +bf1, relu, FC2 (+bf2 via a
    ones-row matmul) and softmax for its 32-batch block.
"""
import json

import numpy as np

import concourse.bass as bass
import concourse.mybir as mybir
import concourse.tile as tile
from concourse.bass_utils import run_bass_kernel_spmd

B, N, F, E = 256, 784, 1, 6272
C, H, N_OUT = 32, 512, 10
NCORE = 8
CPC = C // NCORE      # 4 channels per core in conv2
P = 112               # 784 = 7 * 112
KN = N // P           # 7 node chunks
NPAD = P * NCORE      # 896 padded nodes for the node reshard
NG = 4                # node groups packed into partitions for the mix
NS = P // NG          # 28 nodes per group per core
BPC = B // NCORE      # 32 batch rows per core
HJ = H // 128         # 4 h chunks

f32 = mybir.dt.float32
bf16 = mybir.dt.bfloat16
AF = mybir.ActivationFunctionType
ALU = mybir.AluOpType
AX = mybir.AxisListType


# ---------------------------------------------------------------------------
# BIR post-pass: this walrus build rejects instructions with >1 sync-wait;
# split extras onto standalone EventSemaphore instructions (same engine,
# inserted just before, so the engine stream stalls identically).
def _split_waits(bir: dict, max_waits: int = 1) -> dict:
    n = [0]
    for fn in bir.get("functions", []):
        for blk in fn.get("blocks", []):
            out = []
            for ins in blk.get("instructions", []):
                si = ins.get("sync_info") or {}
                waits = si.get("on_wait") or []
                if len(waits) > max_waits:
                    for w in waits[max_waits:]:
                        n[0] += 1
                        out.append({
                            "name": f"I-waitsplit-{n[0]}",
                            "opcode": "EventSemaphore",
                            "engine": ins["engine"],
                            "ins": [], "outs": [],
                            **({"debug": ins["debug"]} if "debug" in ins else {}),
                            "sync_info": {"on_update": [], "on_wait": [w]},
                        })
                    si = dict(si)
                    si["on_wait"] = waits[:max_waits]
                    ins = dict(ins)
                    ins["sync_info"] = si
                out.append(ins)
            blk["instructions"] = out
    return bir


def _install_wait_splitter(nc):
    orig = nc.to_json_bytes
    nc.to_json_bytes = lambda: json.dumps(_split_waits(json.loads(orig()))).encode()


# ---------------------------------------------------------------------------
def _build_program():
    nc = bass.Bass(num_devices=NCORE)

    at_d = nc.dram_tensor("at", [P, KN * N], bf16, kind="ExternalInput")
    xt_d = nc.dram_tensor("xt", [P, KN * B], bf16, kind="ExternalInput")
    wf1_d = nc.dram_tensor("wf1", [NS * 128, H], bf16, kind="ExternalInput")
    wb_d = nc.dram_tensor("wb", [1, 2 * CPC], f32, kind="ExternalInput")
    w2k_d = nc.dram_tensor("w2k", [128, 128], bf16, kind="ExternalInput")
    b2k_d = nc.dram_tensor("b2k", [128, 1], f32, kind="ExternalInput")
    bf1_d = nc.dram_tensor("bf1", [128, HJ], f32, kind="ExternalInput")
    wf2_d = nc.dram_tensor("wf2", [128, HJ * N_OUT], bf16, kind="ExternalInput")
    bf2_d = nc.dram_tensor("bf2", [1, N_OUT], bf16, kind="ExternalInput")
    out_d = nc.dram_tensor("out", [BPC, N_OUT], f32, kind="ExternalOutput")

    with tile.TileContext(nc) as tc:
        with (
            tc.tile_pool(name="big", bufs=1) as big,
            tc.tile_pool(name="small", bufs=1) as small,
            tc.tile_pool(name="ework", bufs=4) as ework,
            tc.tile_pool(name="rwork", bufs=4) as rwork,
            tc.tile_pool(name="ps_a", bufs=2, space="PSUM") as ps_a,
            tc.tile_pool(name="ps_b", bufs=2, space="PSUM") as ps_b,
            tc.tile_pool(name="ps_fc1", bufs=4, space="PSUM") as ps_fc1,
            tc.tile_pool(name="dram", bufs=1, space="DRAM") as dram,
        ):
            # ---- resident inputs ------------------------------------------
            at_sb = big.tile([P, KN, N], bf16)
            xt_sb = big.tile([P, KN, B], bf16)
            wf1_sb = big.tile([128, NS, H], bf16)
            wb_sb = small.tile([1, 2 * CPC], f32)
            w2k_sb = small.tile([128, 128], bf16)
            b2k_sb = small.tile([128, 1], f32)
            bf1_sb = small.tile([128, HJ], f32)
            wf2_sb = small.tile([128, HJ, N_OUT], bf16)
            bf2_sb = small.tile([1, N_OUT], bf16)
            ones = small.tile([1, 128], bf16)
            ones_f = small.tile([1, 128], f32)

            nc.sync.dma_start(xt_sb[:], xt_d[:].rearrange("p (k b) -> p k b", k=KN))
            nc.sync.dma_start(at_sb[:], at_d[:].rearrange("p (k n) -> p k n", k=KN))
            nc.sync.dma_start(wb_sb[:], wb_d[:])
            nc.sync.dma_start(w2k_sb[:], w2k_d[:])
            nc.sync.dma_start(b2k_sb[:], b2k_d[:])
            nc.sync.dma_start(bf1_sb[:], bf1_d[:])
            nc.sync.dma_start(wf2_sb[:], wf2_d[:].rearrange("p (j o) -> p j o", j=HJ))
            nc.sync.dma_start(bf2_sb[:], bf2_d[:])
            nc.sync.dma_start(wf1_sb[:],
                              wf1_d[:].rearrange("(s p) h -> p s h", p=128))
            nc.vector.memset(ones[:], 1.0)
            nc.vector.memset(ones_f[:], 1.0)

            # ---- broadcast W1/b1 channel scalars across partitions --------
            ps_bc = ps_a.tile([128, 512], f32, tag="psa")
            nc.tensor.matmul(ps_bc[:, 0:2 * CPC], ones_f[0:1, 0:128],
                             wb_sb[:])
            wband = small.tile([128, 2 * CPC], f32)
            nc.vector.tensor_copy(wband[:], ps_bc[:, 0:2 * CPC])

            # ---- conv1 + elu ----------------------------------------------
            h1_sb = big.tile([P, CPC, KN, B], bf16)
            for mc in range(KN):
                o1 = ps_a.tile([128, 512], f32, tag="psa")
                for kc in range(KN):
                    nc.tensor.matmul(
                        o1[0:P, 0:B],
                        at_sb[:, kc, mc * P:(mc + 1) * P],
                        xt_sb[:, kc, :],
                        start=(kc == 0), stop=(kc == KN - 1),
                    )
                for c in range(CPC):
                    sc = wband[0:P, c:c + 1]
                    bi = wband[0:P, CPC + c:CPC + c + 1]
                    e = ework.tile([128, 512], f32)
                    nc.scalar.activation(e[0:P, 0:B], o1[0:P, 0:B], AF.Exp,
                                         bias=bi, scale=sc)
                    r = rwork.tile([128, 512], f32)
                    nc.scalar.activation(r[0:P, 0:B], o1[0:P, 0:B], AF.Relu,
                                         bias=bi, scale=sc)
                    nc.vector.tensor_scalar(e[0:P, 0:B], e[0:P, 0:B],
                                            1.0, -1.0, ALU.min, ALU.add)
                    nc.vector.tensor_tensor(h1_sb[:, c, mc, :], e[0:P, 0:B],
                                            r[0:P, 0:B], ALU.add)

            # ---- conv2 spmm (channel shard) -> out2_sb (bf16) -------------
            out2_sb = big.tile([P, CPC, KN, B], bf16)
            for c in range(CPC):
                for mo in range(KN):
                    o2 = ps_b.tile([P, B], f32, tag="psb")
                    for kc in range(KN):
                        nc.tensor.matmul(
                            o2[:],
                            at_sb[:, kc, mo * P:(mo + 1) * P],
                            h1_sb[:, c, kc, :],
                            start=(kc == 0), stop=(kc == KN - 1),
                        )
                    if (c * KN + mo) % 2 == 0:
                        nc.vector.tensor_copy(out2_sb[:, c, mo, :], o2[:])
                    else:
                        nc.scalar.copy(out2_sb[:, c, mo, :], o2[:])

            # ---- AllToAll: channel shard -> node shard --------------------
            # block j = (4 local channels, nodes [112j,112j+112), all b);
            # node block 7 (nodes 784..895) is zero padding.
            a2a_in = dram.tile([NCORE, CPC * P, B], bf16)
            a2a_out = dram.tile([NCORE, CPC * P, B], bf16)
            zpad = small.tile([P, CPC, B], bf16)
            nc.vector.memset(zpad[:], 0.0)
            for j in range(NCORE):
                dst = a2a_in[j].rearrange("(cl p) b -> p cl b", cl=CPC)
                if j < KN:
                    nc.gpsimd.dma_start(dst, out2_sb[:, :, j, :])
                else:
                    nc.gpsimd.dma_start(dst, zpad[:])

            nc.gpsimd.collective_compute(
                "AllToAll", ALU.bypass,
                replica_groups=[list(range(NCORE))],
                ins=[a2a_in.opt()], outs=[a2a_out.opt()],
            )

            # ---- repack: [(k,cl), p=ng*28+s, b] -> [(ng,c) part, s, b] ----
            r_sb = big.tile([128, NS, B], bf16)
            ap = a2a_out[:].rearrange("k (cl g s) b -> g (k cl) s b",
                                      cl=CPC, g=NG)
            for ng in range(NG):
                nc.gpsimd.dma_start(r_sb[ng * C:(ng + 1) * C, :, :],
                                    ap[ng])

            # ---- W2 mix + b2 + elu -> h2 (bf16) ---------------------------
            h2_sb = big.tile([128, NS, B], bf16)
            for s0 in range(0, NS, 2):
                sw = min(2, NS - s0)
                fw = sw * B
                pm = ps_a.tile([128, 512], f32, tag="psa")
                nc.tensor.matmul(pm[:, 0:fw], w2k_sb[:],
                                 r_sb[:, s0:s0 + sw, :])
                e = ework.tile([128, 512], f32)
                nc.scalar.activation(e[:, 0:fw], pm[:, 0:fw], AF.Exp,
                                     bias=b2k_sb[:, 0:1])
                r = rwork.tile([128, 512], f32)
                nc.scalar.activation(r[:, 0:fw], pm[:, 0:fw], AF.Relu,
                                     bias=b2k_sb[:, 0:1])
                nc.vector.tensor_scalar(e[:, 0:fw], e[:, 0:fw],
                                        1.0, -1.0, ALU.min, ALU.add)
                nc.vector.tensor_tensor(h2_sb[:, s0:s0 + sw, :], e[:, 0:fw],
                                        r[:, 0:fw], ALU.add)

            # ---- FC1 (K shard): z^T partials [512, 256] -------------------
            zps = [ps_fc1.tile([128, B], f32, tag="psfc1", name=f"zp{h}")
                   for h in range(HJ)]
            for s in range(NS):
                for hj in range(HJ):
                    nc.tensor.matmul(
                        zps[hj][:],
                        wf1_sb[:, s, hj * 128:(hj + 1) * 128],
                        h2_sb[:, s, :],
                        start=(s == 0), stop=(s == NS - 1),
                    )

            # ---- ReduceScatter z partials ---------------------------------
            zsb = small.tile([128, HJ, B], f32)
            for hj in range(HJ):
                nc.scalar.copy(zsb[:, hj, :], zps[hj][:])
            rs_in = dram.tile([NCORE, H, BPC], f32)
            rs_out = dram.tile([H, BPC], f32)
            rdst = rs_in[:].rearrange("j h b -> h j b")
            for hj in range(HJ):
                nc.gpsimd.dma_start(
                    rdst[hj * 128:(hj + 1) * 128],
                    zsb[:, hj, :].rearrange("h (j b) -> h j b", j=NCORE),
                )
            nc.gpsimd.collective_compute(
                "ReduceScatter", ALU.add,
                replica_groups=[list(range(NCORE))],
                ins=[rs_in.opt()], outs=[rs_out.opt()],
            )

            # ---- +bf1, relu, FC2, +bf2, softmax ---------------------------
            z_sb = small.tile([128, HJ, BPC], f32)
            zr_sb = small.tile([128, HJ, BPC], bf16)
            for hj in range(HJ):
                nc.sync.dma_start(z_sb[:, hj, :],
                                  rs_out[hj * 128:(hj + 1) * 128, :])
                nc.scalar.activation(zr_sb[:, hj, :], z_sb[:, hj, :], AF.Relu,
                                     bias=bf1_sb[:, hj:hj + 1])

            ps_o = ps_b.tile([BPC, N_OUT], f32, tag="psb")
            for hj in range(HJ):
                nc.tensor.matmul(ps_o[:], zr_sb[:, hj, :], wf2_sb[:, hj, :],
                                 start=(hj == 0), stop=False)
            nc.tensor.matmul(ps_o[:], ones[0:1, 0:BPC], bf2_sb[:],
                             start=False, stop=True)

            mx = small.tile([BPC, 1], f32)
            nc.vector.tensor_reduce(mx[:], ps_o[:], axis=AX.X, op=ALU.max,
                                    negate=True)
            t = small.tile([BPC, N_OUT], f32)
            nc.vector.tensor_scalar(t[:], ps_o[:], mx[0:BPC, 0:1], None, ALU.add)
            ex = small.tile([BPC, N_OUT], f32)
            nc.scalar.activation(ex[:], t[:], AF.Exp)
            sm = small.tile([BPC, 1], f32)
            nc.vector.tensor_reduce(sm[:], ex[:], axis=AX.X, op=ALU.add)
            rc = small.tile([BPC, 1], f32)
            nc.vector.reciprocal(rc[:], sm[:])
            ob = small.tile([BPC, N_OUT], f32)
            nc.vector.tensor_scalar(ob[:], ex[:], rc[0:BPC, 0:1], None, ALU.mult)
            nc.sync.dma_start(out_d[:], ob[:])

    _install_wait_splitter(nc)
    return nc


_NC_CACHE = None


def _get_program():
    global _NC_CACHE
    if _NC_CACHE is None:
        _NC_CACHE = _build_program()
    return _NC_CACHE


# ---------------------------------------------------------------------------
def _prep_inputs(x, edge_row, edge_col, edge_val, W1, b1, W2, b2,
                 Wf1, bf1, Wf2, bf2):
    import ml_dtypes
    f = np.float32
    bf = ml_dtypes.bfloat16
    A = np.zeros((N, N), f)
    np.add.at(A, (np.asarray(edge_row), np.asarray(edge_col)),
              np.asarray(edge_val, f))
    AT = np.ascontiguousarray(A.T)                                  # [m, n]
    at = np.ascontiguousarray(
        AT.reshape(KN, P, N).transpose(1, 0, 2).reshape(P, KN * N)).astype(bf)

    XT = np.ascontiguousarray(np.asarray(x, f)[:, :, 0].T)          # [N, B]
    xt = np.ascontiguousarray(
        XT.reshape(KN, P, B).transpose(1, 0, 2).reshape(P, KN * B)).astype(bf)

    W1 = np.asarray(W1, f); b1 = np.asarray(b1, f)
    W2 = np.asarray(W2, f); b2 = np.asarray(b2, f)
    Wf1 = np.asarray(Wf1, f); bf1 = np.asarray(bf1, f)
    Wf2 = np.asarray(Wf2, f); bf2 = np.asarray(bf2, f)

    # mix weight: lhsT[(ng,c),(ng',c')] = delta(ng,ng') * W2[c,c']
    w2k = np.kron(np.eye(NG, dtype=f), W2).astype(bf)               # [128,128]
    b2k = np.tile(b2, NG).reshape(128, 1).astype(f)

    # FC1: core k's K-chunk s holds flat rows (n=112k+ng*28+s)*C + c' at
    # partition p = ng*C + c'; rows for pad nodes (n >= 784) are zero.
    Wf1_pad = np.zeros((NPAD, C, H), f)
    Wf1_pad[:N] = Wf1.reshape(N, C, H)

    bf1_l = np.ascontiguousarray(bf1.reshape(HJ, 128).T)            # [128, HJ]
    wf2_l = np.ascontiguousarray(
        Wf2.reshape(HJ, 128, N_OUT).transpose(1, 0, 2).reshape(
            128, HJ * N_OUT)).astype(bf)
    bf2_l = bf2.reshape(1, N_OUT).astype(bf)

    in_maps = []
    for k in range(NCORE):
        wb = np.concatenate([W1[0, k * CPC:(k + 1) * CPC],
                             b1[k * CPC:(k + 1) * CPC]]).reshape(1, 2 * CPC)
        # [NG, NS, C, H] -> chunk s, partition (ng, c')
        wk = Wf1_pad[k * P:(k + 1) * P].reshape(NG, NS, C, H)
        wf1_l = np.ascontiguousarray(
            wk.transpose(1, 0, 2, 3).reshape(NS * 128, H)).astype(bf)
        in_maps.append({
            "at": at, "xt": xt, "wf1": wf1_l,
            "wb": np.ascontiguousarray(wb.astype(f)),
            "w2k": w2k, "b2k": b2k,
            "bf1": bf1_l, "wf2": wf2_l, "bf2": bf2_l,
        })
    return in_maps


def kernel(x, edge_row, edge_col, edge_val, W1, b1, W2, b2,
           Wf1, bf1, Wf2, bf2, **kw):
    nc = _get_program()
    in_maps = _prep_inputs(x, edge_row, edge_col, edge_val, W1, b1, W2, b2,
                           Wf1, bf1, Wf2, bf2)
    res = run_bass_kernel_spmd(nc, in_maps, list(range(NCORE)), **kw)
    out = np.concatenate([res.results[k]["out"] for k in range(NCORE)], axis=0)
    if kw.get("trace"):
        kernel.last_exec_time_ns = res.exec_time_ns
    return out.astype(np.float32)


# revision 11
# speedup vs baseline: 2.2369x; 1.0095x over previous
"""GNN message-passing net on 8 Trainium2 cores.

Reference: x:[256,784,1] -> h1 = elu(spmm(x)@W1+b1) -> h2 = elu(spmm(h1)@W2+b2)
-> flat[B, N*C] -> relu(flat@Wf1+bf1) -> softmax(z@Wf2+bf2).

Strategy (all matmul operands bf16, fp32 PSUM accumulation):
  * Densify the sparse filter A (784x784, ~1% nz) on the host; spmm becomes
    dense matmuls on the PE array.
  * F=1 makes conv1 an outer product: out1 = A @ X^T [784,256] shared by all
    channels; h1_c = elu(W1[c]*out1+b1[c]) via ACT Exp/Relu with fused
    per-partition scale/bias + 2 DVE ops (elu(t)=min(exp(t),1)+relu(t)-1).
  * Conv2 spmm channel-sharded: core k computes out2_c = A @ h1_c for
    channels 4k..4k+3, full batch (free=256).
  * AllToAll reshards channel->node: core j receives all 32 pre-mix channels
    for nodes [112j, 112j+112) (core 7 gets zero-padded nodes 784..895),
    packed [(ng,c) partitions, s, b] with its nodes split 4x28.
  * W2 channel mix as a 128x128 stationary kron(I4,W2) matmul; +b2, elu.
  * FC1 stays K-sharded: core k holds Wf1 rows for its nodes (zero rows for
    pad nodes), 28 K-chunks x 4 h-chunks, free=256.  z^T partials [512,256]
    are ReduceScattered; each core then does +bf1, relu, FC2 (+bf2 via a
    ones-row matmul) and softmax for its 32-batch block.
"""
import json

import numpy as np

import concourse.bass as bass
import concourse.mybir as mybir
import concourse.tile as tile
from concourse.bass_utils import run_bass_kernel_spmd

B, N, F, E = 256, 784, 1, 6272
C, H, N_OUT = 32, 512, 10
NCORE = 8
CPC = C // NCORE      # 4 channels per core in conv2
P = 112               # 784 = 7 * 112
KN = N // P           # 7 node chunks
NPAD = P * NCORE      # 896 padded nodes for the node reshard
NG = 4                # node groups packed into partitions for the mix
NS = P // NG          # 28 nodes per group per core
BPC = B // NCORE      # 32 batch rows per core
HJ = H // 128         # 4 h chunks

f32 = mybir.dt.float32
bf16 = mybir.dt.bfloat16
AF = mybir.ActivationFunctionType
ALU = mybir.AluOpType
AX = mybir.AxisListType


# ---------------------------------------------------------------------------
# BIR post-pass: this walrus build rejects instructions with >1 sync-wait;
# split extras onto standalone EventSemaphore instructions (same engine,
# inserted just before, so the engine stream stalls identically).
def _split_waits(bir: dict, max_waits: int = 1) -> dict:
    n = [0]
    for fn in bir.get("functions", []):
        for blk in fn.get("blocks", []):
            out = []
            for ins in blk.get("instructions", []):
                si = ins.get("sync_info") or {}
                waits = si.get("on_wait") or []
                if len(waits) > max_waits:
                    for w in waits[max_waits:]:
                        n[0] += 1
                        out.append({
                            "name": f"I-waitsplit-{n[0]}",
                            "opcode": "EventSemaphore",
                            "engine": ins["engine"],
                            "ins": [], "outs": [],
                            **({"debug": ins["debug"]} if "debug" in ins else {}),
                            "sync_info": {"on_update": [], "on_wait": [w]},
                        })
                    si = dict(si)
                    si["on_wait"] = waits[:max_waits]
                    ins = dict(ins)
                    ins["sync_info"] = si
                out.append(ins)
            blk["instructions"] = out
    return bir


def _install_wait_splitter(nc):
    orig = nc.to_json_bytes
    nc.to_json_bytes = lambda: json.dumps(_split_waits(json.loads(orig()))).encode()


# ---------------------------------------------------------------------------
def _build_program():
    nc = bass.Bass(num_devices=NCORE)

    at_d = nc.dram_tensor("at", [P, KN * N], bf16, kind="ExternalInput")
    xt_d = nc.dram_tensor("xt", [P, KN * B], bf16, kind="ExternalInput")
    wf1_d = nc.dram_tensor("wf1", [NS * 128, H], bf16, kind="ExternalInput")
    wb_d = nc.dram_tensor("wb", [1, 2 * CPC], f32, kind="ExternalInput")
    w2k_d = nc.dram_tensor("w2k", [128, 128], bf16, kind="ExternalInput")
    b2k_d = nc.dram_tensor("b2k", [128, 1], f32, kind="ExternalInput")
    bf1_d = nc.dram_tensor("bf1", [128, HJ], f32, kind="ExternalInput")
    wf2_d = nc.dram_tensor("wf2", [128, HJ * N_OUT], bf16, kind="ExternalInput")
    bf2_d = nc.dram_tensor("bf2", [1, N_OUT], bf16, kind="ExternalInput")
    out_d = nc.dram_tensor("out", [BPC, N_OUT], f32, kind="ExternalOutput")

    with tile.TileContext(nc) as tc:
        with (
            tc.tile_pool(name="big", bufs=1) as big,
            tc.tile_pool(name="small", bufs=1) as small,
            tc.tile_pool(name="ework", bufs=4) as ework,
            tc.tile_pool(name="rwork", bufs=4) as rwork,
            tc.tile_pool(name="ps_a", bufs=2, space="PSUM") as ps_a,
            tc.tile_pool(name="ps_b", bufs=2, space="PSUM") as ps_b,
            tc.tile_pool(name="ps_fc1", bufs=4, space="PSUM") as ps_fc1,
            tc.tile_pool(name="dram", bufs=1, space="DRAM") as dram,
        ):
            # ---- resident inputs ------------------------------------------
            at_sb = big.tile([P, KN, N], bf16)
            xt_sb = big.tile([P, KN, B], bf16)
            wf1_sb = big.tile([128, NS, H], bf16)
            wb_sb = small.tile([1, 2 * CPC], f32)
            w2k_sb = small.tile([128, 128], bf16)
            b2k_sb = small.tile([128, 1], f32)
            bf1_sb = small.tile([128, HJ], f32)
            wf2_sb = small.tile([128, HJ, N_OUT], bf16)
            bf2_sb = small.tile([1, N_OUT], bf16)
            ones = small.tile([1, 128], bf16)
            ones_f = small.tile([1, 128], f32)

            xt_ap = xt_d[:].rearrange("p (k b) -> p k b", k=KN)
            at_ap = at_d[:].rearrange("p (k n) -> p k n", k=KN)
            for kc in range(KN):
                nc.sync.dma_start(xt_sb[:, kc, :], xt_ap[:, kc, :])
                nc.sync.dma_start(at_sb[:, kc, :], at_ap[:, kc, :])
            nc.sync.dma_start(wb_sb[:], wb_d[:])
            nc.sync.dma_start(w2k_sb[:], w2k_d[:])
            nc.sync.dma_start(b2k_sb[:], b2k_d[:])
            nc.sync.dma_start(bf1_sb[:], bf1_d[:])
            nc.sync.dma_start(wf2_sb[:], wf2_d[:].rearrange("p (j o) -> p j o", j=HJ))
            nc.sync.dma_start(bf2_sb[:], bf2_d[:])
            nc.sync.dma_start(wf1_sb[:],
                              wf1_d[:].rearrange("(s p) h -> p s h", p=128))
            nc.vector.memset(ones[:], 1.0)
            nc.vector.memset(ones_f[:], 1.0)

            # ---- broadcast W1/b1 channel scalars across partitions --------
            ps_bc = ps_a.tile([128, 512], f32, tag="psa")
            nc.tensor.matmul(ps_bc[:, 0:2 * CPC], ones_f[0:1, 0:128],
                             wb_sb[:])
            wband = small.tile([128, 2 * CPC], f32)
            nc.vector.tensor_copy(wband[:], ps_bc[:, 0:2 * CPC])

            # ---- conv1 + elu ----------------------------------------------
            h1_sb = big.tile([P, CPC, KN, B], bf16)
            for mc in range(KN):
                o1 = ps_a.tile([128, 512], f32, tag="psa")
                for kc in range(KN):
                    nc.tensor.matmul(
                        o1[0:P, 0:B],
                        at_sb[:, kc, mc * P:(mc + 1) * P],
                        xt_sb[:, kc, :],
                        start=(kc == 0), stop=(kc == KN - 1),
                    )
                for c in range(CPC):
                    sc = wband[0:P, c:c + 1]
                    bi = wband[0:P, CPC + c:CPC + c + 1]
                    e = ework.tile([128, 512], f32)
                    nc.scalar.activation(e[0:P, 0:B], o1[0:P, 0:B], AF.Exp,
                                         bias=bi, scale=sc)
                    r = rwork.tile([128, 512], f32)
                    nc.scalar.activation(r[0:P, 0:B], o1[0:P, 0:B], AF.Relu,
                                         bias=bi, scale=sc)
                    nc.vector.tensor_scalar(e[0:P, 0:B], e[0:P, 0:B],
                                            1.0, -1.0, ALU.min, ALU.add)
                    nc.vector.tensor_tensor(h1_sb[:, c, mc, :], e[0:P, 0:B],
                                            r[0:P, 0:B], ALU.add)

            # ---- AllToAll: channel shard -> node shard --------------------
            # block j = (4 local channels, nodes [112j,112j+112), all b);
            # node block 7 (nodes 784..895) is zero padding.
            r_sb = big.tile([128, NS, B], bf16)
            a2a_in = dram.tile([NCORE, CPC * P, B], bf16)
            a2a_out = dram.tile([NCORE, CPC * P, B], bf16)
            zpad = small.tile([P, CPC, B], bf16)
            nc.vector.memset(zpad[:], 0.0)

            def a2a_all():
                for j in range(NCORE):
                    dst = a2a_in[j].rearrange("(cl p) b -> p cl b", cl=CPC)
                    if j < KN:
                        nc.gpsimd.dma_start(dst, out2_sb[:, :, j, :])
                    else:
                        nc.gpsimd.dma_start(dst, zpad[:])
                nc.gpsimd.collective_compute(
                    "AllToAll", ALU.bypass,
                    replica_groups=[list(range(NCORE))],
                    ins=[a2a_in.opt()], outs=[a2a_out.opt()],
                )
                ap = a2a_out[:].rearrange("k (cl g s) b -> g (k cl) s b",
                                          cl=CPC, g=NG)
                for ng in range(NG):
                    nc.gpsimd.dma_start(r_sb[ng * C:(ng + 1) * C, :, :],
                                        ap[ng])

            # ---- conv2 spmm (channel shard) -> out2_sb (bf16) -------------
            out2_sb = big.tile([P, CPC, KN, B], bf16)
            for c in range(CPC):
                for mo in range(KN):
                    o2 = ps_b.tile([P, B], f32, tag="psb")
                    for kc in range(KN):
                        nc.tensor.matmul(
                            o2[:],
                            at_sb[:, kc, mo * P:(mo + 1) * P],
                            h1_sb[:, c, kc, :],
                            start=(kc == 0), stop=(kc == KN - 1),
                        )
                    if (c * KN + mo) % 2 == 0:
                        nc.vector.tensor_copy(out2_sb[:, c, mo, :], o2[:])
                    else:
                        nc.scalar.copy(out2_sb[:, c, mo, :], o2[:])
                if c == CPC - 1:
                    a2a_all()

            # ---- W2 mix + b2 + elu -> h2 (bf16) ---------------------------
            h2_sb = big.tile([128, NS, B], bf16)
            for s0 in range(0, NS, 2):
                sw = min(2, NS - s0)
                fw = sw * B
                pm = ps_a.tile([128, 512], f32, tag="psa")
                nc.tensor.matmul(pm[:, 0:fw], w2k_sb[:],
                                 r_sb[:, s0:s0 + sw, :])
                e = ework.tile([128, 512], f32)
                nc.scalar.activation(e[:, 0:fw], pm[:, 0:fw], AF.Exp,
                                     bias=b2k_sb[:, 0:1])
                r = rwork.tile([128, 512], f32)
                nc.scalar.activation(r[:, 0:fw], pm[:, 0:fw], AF.Relu,
                                     bias=b2k_sb[:, 0:1])
                nc.vector.tensor_scalar(e[:, 0:fw], e[:, 0:fw],
                                        1.0, -1.0, ALU.min, ALU.add)
                nc.vector.tensor_tensor(h2_sb[:, s0:s0 + sw, :], e[:, 0:fw],
                                        r[:, 0:fw], ALU.add)

            # ---- FC1 (K shard): z^T partials [512, 256] -------------------
            zps = [ps_fc1.tile([128, B], f32, tag="psfc1", name=f"zp{h}")
                   for h in range(HJ)]
            for s in range(NS):
                for hj in range(HJ):
                    nc.tensor.matmul(
                        zps[hj][:],
                        wf1_sb[:, s, hj * 128:(hj + 1) * 128],
                        h2_sb[:, s, :],
                        start=(s == 0), stop=(s == NS - 1),
                    )

            # ---- ReduceScatter z partials ---------------------------------
            zsb = small.tile([128, HJ, B], f32)
            for hj in range(HJ):
                nc.scalar.copy(zsb[:, hj, :], zps[hj][:])
            rs_in = dram.tile([NCORE, H, BPC], f32)
            rs_out = dram.tile([H, BPC], f32)
            rdst = rs_in[:].rearrange("j h b -> h j b")
            for hj in range(HJ):
                nc.gpsimd.dma_start(
                    rdst[hj * 128:(hj + 1) * 128],
                    zsb[:, hj, :].rearrange("h (j b) -> h j b", j=NCORE),
                )
            nc.gpsimd.collective_compute(
                "ReduceScatter", ALU.add,
                replica_groups=[list(range(NCORE))],
                ins=[rs_in.opt()], outs=[rs_out.opt()],
            )

            # ---- +bf1, relu, FC2, +bf2, softmax ---------------------------
            z_sb = small.tile([128, HJ, BPC], f32)
            zr_sb = small.tile([128, HJ, BPC], bf16)
            for hj in range(HJ):
                nc.sync.dma_start(z_sb[:, hj, :],
                                  rs_out[hj * 128:(hj + 1) * 128, :])
                nc.scalar.activation(zr_sb[:, hj, :], z_sb[:, hj, :], AF.Relu,
                                     bias=bf1_sb[:, hj:hj + 1])

            ps_o = ps_b.tile([BPC, N_OUT], f32, tag="psb")
            for hj in range(HJ):
                nc.tensor.matmul(ps_o[:], zr_sb[:, hj, :], wf2_sb[:, hj, :],
                                 start=(hj == 0), stop=False)
            nc.tensor.matmul(ps_o[:], ones[0:1, 0:BPC], bf2_sb[:],
                             start=False, stop=True)

            mx = small.tile([BPC, 1], f32)
            nc.vector.tensor_reduce(mx[:], ps_o[:], axis=AX.X, op=ALU.max,
                                    negate=True)
            t = small.tile([BPC, N_OUT], f32)
            nc.vector.tensor_scalar(t[:], ps_o[:], mx[0:BPC, 0:1], None, ALU.add)
            ex = small.tile([BPC, N_OUT], f32)
            nc.scalar.activation(ex[:], t[:], AF.Exp)
            sm = small.tile([BPC, 1], f32)
            nc.vector.tensor_reduce(sm[:], ex[:], axis=AX.X, op=ALU.add)
            rc = small.tile([BPC, 1], f32)
            nc.vector.reciprocal(rc[:], sm[:])
            ob = small.tile([BPC, N_OUT], f32)
            nc.vector.tensor_scalar(ob[:], ex[:], rc[0:BPC, 0:1], None, ALU.mult)
            nc.sync.dma_start(out_d[:], ob[:])

    _install_wait_splitter(nc)
    return nc


_NC_CACHE = None


def _get_program():
    global _NC_CACHE
    if _NC_CACHE is None:
        _NC_CACHE = _build_program()
    return _NC_CACHE


# ---------------------------------------------------------------------------
def _prep_inputs(x, edge_row, edge_col, edge_val, W1, b1, W2, b2,
                 Wf1, bf1, Wf2, bf2):
    import ml_dtypes
    f = np.float32
    bf = ml_dtypes.bfloat16
    A = np.zeros((N, N), f)
    np.add.at(A, (np.asarray(edge_row), np.asarray(edge_col)),
              np.asarray(edge_val, f))
    AT = np.ascontiguousarray(A.T)                                  # [m, n]
    at = np.ascontiguousarray(
        AT.reshape(KN, P, N).transpose(1, 0, 2).reshape(P, KN * N)).astype(bf)

    XT = np.ascontiguousarray(np.asarray(x, f)[:, :, 0].T)          # [N, B]
    xt = np.ascontiguousarray(
        XT.reshape(KN, P, B).transpose(1, 0, 2).reshape(P, KN * B)).astype(bf)

    W1 = np.asarray(W1, f); b1 = np.asarray(b1, f)
    W2 = np.asarray(W2, f); b2 = np.asarray(b2, f)
    Wf1 = np.asarray(Wf1, f); bf1 = np.asarray(bf1, f)
    Wf2 = np.asarray(Wf2, f); bf2 = np.asarray(bf2, f)

    # mix weight: lhsT[(ng,c),(ng',c')] = delta(ng,ng') * W2[c,c']
    w2k = np.kron(np.eye(NG, dtype=f), W2).astype(bf)               # [128,128]
    b2k = np.tile(b2, NG).reshape(128, 1).astype(f)

    # FC1: core k's K-chunk s holds flat rows (n=112k+ng*28+s)*C + c' at
    # partition p = ng*C + c'; rows for pad nodes (n >= 784) are zero.
    Wf1_pad = np.zeros((NPAD, C, H), f)
    Wf1_pad[:N] = Wf1.reshape(N, C, H)

    bf1_l = np.ascontiguousarray(bf1.reshape(HJ, 128).T)            # [128, HJ]
    wf2_l = np.ascontiguousarray(
        Wf2.reshape(HJ, 128, N_OUT).transpose(1, 0, 2).reshape(
            128, HJ * N_OUT)).astype(bf)
    bf2_l = bf2.reshape(1, N_OUT).astype(bf)

    in_maps = []
    for k in range(NCORE):
        wb = np.concatenate([W1[0, k * CPC:(k + 1) * CPC],
                             b1[k * CPC:(k + 1) * CPC]]).reshape(1, 2 * CPC)
        # [NG, NS, C, H] -> chunk s, partition (ng, c')
        wk = Wf1_pad[k * P:(k + 1) * P].reshape(NG, NS, C, H)
        wf1_l = np.ascontiguousarray(
            wk.transpose(1, 0, 2, 3).reshape(NS * 128, H)).astype(bf)
        in_maps.append({
            "at": at, "xt": xt, "wf1": wf1_l,
            "wb": np.ascontiguousarray(wb.astype(f)),
            "w2k": w2k, "b2k": b2k,
            "bf1": bf1_l, "wf2": wf2_l, "bf2": bf2_l,
        })
    return in_maps


def kernel(x, edge_row, edge_col, edge_val, W1, b1, W2, b2,
           Wf1, bf1, Wf2, bf2, **kw):
    nc = _get_program()
    in_maps = _prep_inputs(x, edge_row, edge_col, edge_val, W1, b1, W2, b2,
                           Wf1, bf1, Wf2, bf2)
    res = run_bass_kernel_spmd(nc, in_maps, list(range(NCORE)), **kw)
    out = np.concatenate([res.results[k]["out"] for k in range(NCORE)], axis=0)
    if kw.get("trace"):
        kernel.last_exec_time_ns = res.exec_time_ns
    return out.astype(np.float32)
